# revision 1
# baseline (speedup 1.0000x reference)
"""Causal attention (B=4, T=2048, D=1024) on 8 TRN2 NeuronCores.

Fused-M formulation: since scores = (x_q Wq)(x Wk)^T = x_q (Wq Wk^T) x^T,
each core computes M2 = Wq Wk^T once and never materializes K. Likewise
O = P V = (P x) Wv, so V is never materialized. This removes the
duplicated K/V projection work that batch-split cores otherwise repeat
(both cores of a batch need all keys).

Sharding: core c = (batch b = c//2, half h = c%2). Each core owns 16
blocks of 64 query rows of one batch, packed two per slot (8 static
slots with caps [16,14,...,2] k-tiles; q-cols 0:64 = "even" block
needing cap k-tiles, 64:128 = "odd" block needing cap-1). With 64-row
blocks the per-slot caps match BOTH cores' causal needs exactly (zero
overhang): h=0 pair p owns 64-blocks (31-4p, 29-4p), h=1 (30-4p,
28-4p). Only the two diagonal sub-units per slot are masked, with
per-core tril mask data from the host; the odd block's unused tail of
the last k-tile unit is memset to zero so the 128-wide row-sum and ZT
reads stay exact.

Math per core (matmul inputs bf16, fp32 PSUM):
  M2[i,j] = sum_e WqT[e,i] WkT[e,j]
  GT[j,q] = sum_i M2[i,j] xqT[i,q]
  ST[k,q] = sum_j xkT[j,k] GT[j,q]          (per slot, k < cap*128)
  PT[k,q] = exp(ST/sqrt(D)) * mask          (no max-sub: logits ~N(0,1))
  ZT[j,q] = sum_k xnat[k,j] PT[k,q];  sum[q] = sum_k PT[k,q]
  O[q,e]  = (sum_j ZT[j,q] Wv[j,e]) / sum[q]
"""

import os
import numpy as np
import ml_dtypes

EXP = os.environ.get("KEXP", "")

import concourse.bacc as bacc
import concourse.mybir as mybir
import concourse.tile as tile
from concourse.bass_utils import run_bass_kernel_spmd

BF16 = mybir.dt.bfloat16
F32 = mybir.dt.float32

B, T, D = 4, 2048, 1024
P = 128
NQ = 1024                      # query rows per core
DT = D // P                    # 8 tiles along a 1024 dim
KT_N = T // P                  # 16 k-tiles
SLOT_CAPS = [16, 14, 12, 10, 8, 6, 4, 2]
# Each slot holds a PAIR of 64-row blocks: q-cols [0:64) = "even" block
# (needs cap k-tiles), [64:128) = "odd" block (needs cap-1). With 64-row
# blocks both cores' needs match the caps exactly (zero overhang):
#   h=0 pair p: 64-blocks (31-4p, 29-4p);  h=1: (30-4p, 28-4p)
B64 = {0: [(31 - 4 * p, 29 - 4 * p) for p in range(8)],
       1: [(30 - 4 * p, 28 - 4 * p) for p in range(8)]}
OFF = [0]
for _c in SLOT_CAPS:
    OFF.append(OFF[-1] + _c)
NUNIT = OFF[-1]                # 72
SCALE = 1.0 / np.sqrt(np.float32(D))

_NC_CACHE = None


def build_nc(repeat=1, hw_loop=True):
    """repeat>1 replays the compute pipeline (M2..out) that many times,
    reusing the loaded inputs — used only for differential wall-clock
    timing of the on-device execution (identical I/O footprint)."""
    nc = bacc.Bacc("TRN2", target_bir_lowering=False, debug=False,
                   enable_asserts=False, enable_partition_id=False)

    WqT = nc.dram_tensor("WqT", [D, D], BF16, kind="ExternalInput").ap()
    WkT = nc.dram_tensor("WkT", [D, D], BF16, kind="ExternalInput").ap()
    Wv = nc.dram_tensor("Wv", [D, D], BF16, kind="ExternalInput").ap()
    xqT = nc.dram_tensor("xqT", [D, NQ], BF16, kind="ExternalInput").ap()
    xkT = nc.dram_tensor("xkT", [D, T], BF16, kind="ExternalInput").ap()
    xnat = nc.dram_tensor("xnat", [T, D], BF16, kind="ExternalInput").ap()
    masks = nc.dram_tensor("masks", [P, 16 * 64], BF16, kind="ExternalInput").ap()
    out_d = nc.dram_tensor("out", [NQ, D], BF16, kind="ExternalOutput").ap()
    if "dbg" in EXP:
        dbg_m2 = nc.dram_tensor("dbg_m2", [P, DT, D], BF16, kind="ExternalOutput").ap()
        dbg_gt = nc.dram_tensor("dbg_gt", [P, DT, NQ], BF16, kind="ExternalOutput").ap()
        dbg_pt = nc.dram_tensor("dbg_pt", [P, NUNIT, P], BF16, kind="ExternalOutput").ap()
        dbg_zt = nc.dram_tensor("dbg_zt", [P, 2, DT, P], BF16, kind="ExternalOutput").ap()

    with tile.TileContext(nc) as tc:
        with tc.tile_pool(name="sb", bufs=1) as sb, \
             tc.tile_pool(name="ps", bufs=1, space="PSUM") as ps:

            # ---- stage A: load inputs (et-interleaved weights first) ----
            wq_s = sb.tile([P, DT, D], BF16, tag="wq", bufs=1)
            wk_s = sb.tile([P, DT, D], BF16, tag="wk", bufs=1)
            _wqr = WqT.rearrange("(et p) i -> p et i", p=P)
            _wkr = WkT.rearrange("(et p) j -> p et j", p=P)
            # pass 1 of M2 needs only the ic 0..3 half of each WqT tile; defer
            # the other half so the PE's first matmul starts sooner
            # first two transfers issue from different SEQ engines so their
            # fixed HWDGE/DGE launch latencies overlap
            nc.sync.dma_start(out=wq_s[:, 0, 0:512], in_=_wqr[:, 0, 0:512])
            nc.scalar.dma_start(out=wk_s[:, 0, 0:512], in_=_wkr[:, 0, 0:512])
            nc.scalar.dma_start(out=wk_s[:, 0, 512:1024], in_=_wkr[:, 0, 512:1024])
            for et in range(1, DT):
                nc.sync.dma_start(out=wq_s[:, et, 0:512], in_=_wqr[:, et, 0:512])
                nc.sync.dma_start(out=wk_s[:, et, :], in_=_wkr[:, et, :])
            for et in range(DT):
                nc.sync.dma_start(out=wq_s[:, et, 512:1024],
                                  in_=_wqr[:, et, 512:1024])
            xqT_s = sb.tile([P, DT, NQ], BF16, tag="xq", bufs=1)
            nc.sync.dma_start(out=xqT_s, in_=xqT.rearrange("(ic p) q -> p ic q", p=P))
            xkT_s = sb.tile([P, DT, T], BF16, tag="xk", bufs=1)
            nc.sync.dma_start(out=xkT_s, in_=xkT.rearrange("(jc p) t -> p jc t", p=P))
            xnat_s = sb.tile([P, KT_N, D], BF16, tag="xn", bufs=1)
            nc.sync.dma_start(out=xnat_s, in_=xnat.rearrange("(kt p) j -> p kt j", p=P))
            wv_s = sb.tile([P, DT, D], BF16, tag="wv", bufs=1)
            nc.sync.dma_start(out=wv_s, in_=Wv.rearrange("(jc p) e -> p jc e", p=P))
            masks_s = sb.tile([P, 16 * 64], BF16, tag="mask", bufs=1)
            nc.sync.dma_start(out=masks_s, in_=masks)
            ones_s = sb.tile([P, 1], BF16, tag="ones", bufs=1)
            nc.vector.memset(ones_s, 1.0)

            import contextlib
            n_emit = 1 if hw_loop else repeat
            _loop = (tc.For_i(0, repeat, 1) if (hw_loop and repeat > 1)
                     else contextlib.nullcontext())
            with _loop:
              for rep in range(n_emit):
                r = f"_{rep}" if n_emit > 1 else ""
                m2_s = sb.tile([P, DT, D], BF16, tag="m2", bufs=1, name=f"m2{r}")
                gt_s = sb.tile([P, DT, NQ], BF16, tag="gt", bufs=1, name=f"gt{r}")
                pt_s = sb.tile([P, NUNIT, P], BF16, tag="pt", bufs=1, name=f"pt{r}")

                # ---- stage B: M2 = Wq Wk^T, et-outer over 8 chains/pass ----
                # chains = (ic within pass 0..3) x (j-half 0..1), mapped onto
                # 3x "half" + 1x "sum" + 2x "big"(2 slices) psum tiles.
                def copy_act(out, in_):
                    nc.scalar.copy(out=out, in_=in_)

                def copy_dve(out, in_):
                    nc.vector.tensor_copy(out=out, in_=in_)

                cp_eng = [copy_dve, copy_act]
                for p_i in range(2):
                    halves = [ps.tile([P, 512], F32, tag="half", bufs=3,
                                      name=f"m2h{r}_{p_i}_{i}") for i in range(3)]
                    sumt = ps.tile([P, 512], F32, tag="sum", bufs=1,
                                   name=f"m2s{r}_{p_i}")
                    bigs = [ps.tile([P, 1024], F32, tag="big", bufs=2,
                                    name=f"m2b{r}_{p_i}_{i}") for i in range(2)]
                    sl = (halves[0], halves[1], halves[2], sumt[:, 0:512],
                          bigs[0][:, 0:512], bigs[0][:, 512:1024],
                          bigs[1][:, 0:512], bigs[1][:, 512:1024])
                    for et in range(DT):
                        # h=0 chains first on et==0: the wk half-tiles land in
                        # DMA order [wq lo-half, wk all], so h=1 data is ready
                        # slightly later
                        order = (0, 2, 4, 6, 1, 3, 5, 7) if (p_i == 0 and et == 0) \
                            else range(8)
                        for c in order:
                            ic, h = 4 * p_i + c // 2, c % 2
                            nc.tensor.matmul(
                                sl[c], wq_s[:, et, ic * P:(ic + 1) * P],
                                wk_s[:, et, h * 512:(h + 1) * 512],
                                start=(et == 0), stop=(et == DT - 1))
                    # c0/c1 (the "half"-tag tiles) copied first: their chains
                    # stop earliest in the et7 round, and the next stage's
                    # first psum request rotates onto the half tag
                    for c in range(8):
                        ic, h = 4 * p_i + c // 2, c % 2
                        cp_eng[c % 2](
                            out=m2_s[:, ic, h * 512:(h + 1) * 512], in_=sl[c])

                # ---- stage C: GT = M2^T x_q^T ----
                # jc0 runs on half-tag tiles: those free ~1.5us before the
                # bigs after M2's final pass, so the PE never stalls
                for jc in range(DT):
                    if jc == 0:
                        hts = [ps.tile([P, 512], F32, tag="half", bufs=3,
                                       name=f"gt{r}_h{i}") for i in range(2)]
                        gsl = lambda ch: hts[ch]
                    else:
                        bt = ps.tile([P, 1024], F32, tag="big", bufs=2,
                                     name=f"gt{r}_{jc}")
                        gsl = lambda ch: bt[:, ch * 512:(ch + 1) * 512]
                    for ic in range(DT):
                        for ch in range(2):
                            nc.tensor.matmul(
                                gsl(ch),
                                m2_s[:, ic, jc * P:(jc + 1) * P],
                                xqT_s[:, ic, ch * 512:(ch + 1) * 512],
                                start=(ic == 0), stop=(ic == DT - 1))
                    for ch in range(2):
                        cp_eng[ch](
                            out=gt_s[:, jc, ch * 512:(ch + 1) * 512],
                            in_=gsl(ch))

                # ---- stages D-F: per-slot ST -> exp/mask -> ZT/sums -> O ----
                sum_ps = ps.tile([P, 512], F32, tag="sum", bufs=1,
                                 name=f"sums{r}")
                zt_s = [sb.tile([P, DT, P], BF16, tag="zt", bufs=2,
                                name=f"zt{r}_{i}") for i in range(2)]
                o_sb = [sb.tile([P, D], BF16, tag="osb", bufs=2,
                                name=f"o{r}_{i}") for i in range(2)]
                recip = [sb.tile([P, 1], F32, tag="recip", bufs=2,
                                 name=f"rc{r}_{i}") for i in range(2)]

                def emit_st(s):
                    cap = SLOT_CAPS[s]
                    for g in range((cap + 3) // 4):
                        ht = ps.tile([P, 512], F32, tag="half", bufs=3,
                                     name=f"st{r}_{s}_{g}")
                        kts = range(4 * g, min(cap, 4 * g + 4))
                        for kt in kts:
                            c0 = (kt % 4) * P
                            # last k-tile: only the even 64-block's q-cols
                            # reach this far (odd block needs cap-1 k-tiles)
                            w_kt = 64 if kt == cap - 1 else P
                            for jc in range(DT):
                                nc.tensor.matmul(
                                    ht[:, c0:c0 + w_kt],
                                    xkT_s[:, jc, kt * P:(kt + 1) * P],
                                    gt_s[:, jc, s * P:s * P + w_kt],
                                    start=(jc == 0), stop=(jc == DT - 1))
                        # one wide exp per psum tile (after ALL its matmuls:
                        # avoids PE write-after-Act-read stalls on the tile);
                        # the tile with the slot's last k-tile gets a second
                        # 64-wide exp for the partial unit
                        u0 = OFF[s] + 4 * g
                        n_full = len(kts) - (1 if w_kt == 64 else 0)
                        if n_full:
                            nc.scalar.activation(
                                out=pt_s[:, u0:u0 + n_full, :],
                                in_=ht[:, 0:n_full * P],
                                func=mybir.ActivationFunctionType.Exp,
                                scale=float(SCALE))
                        if w_kt == 64:
                            nc.scalar.activation(
                                out=pt_s[:, u0 + n_full, 0:64],
                                in_=ht[:, n_full * P:n_full * P + 64],
                                func=mybir.ActivationFunctionType.Exp,
                                scale=float(SCALE))
                    # zero the never-written odd tail of the last unit (sums
                    # read the full 128-wide unit)
                    nc.vector.memset(pt_s[:, OFF[s] + cap - 1, 64:128], 0.0)
                    if "nomask" not in EXP:
                        # diagonal masks: odd block on unit cap-2 cols[64:],
                        # even block on unit cap-1 cols[:64]
                        nc.vector.tensor_mul(
                            out=pt_s[:, OFF[s] + cap - 2, 64:128],
                            in0=pt_s[:, OFF[s] + cap - 2, 64:128],
                            in1=masks_s[:, (2 * s + 1) * 64:(2 * s + 2) * 64])
                        nc.vector.tensor_mul(
                            out=pt_s[:, OFF[s] + cap - 1, 0:64],
                            in0=pt_s[:, OFF[s] + cap - 1, 0:64],
                            in1=masks_s[:, 2 * s * 64:(2 * s + 1) * 64])

                def emit_zt(s, use_halves=False):
                    cap = SLOT_CAPS[s]
                    if use_halves:
                        hts = [ps.tile([P, 512], F32, tag="half", bufs=3,
                                       name=f"zt{r}_{s}_{i}") for i in range(2)]
                        zsl = lambda jc: hts[jc // 4][:, (jc % 4) * P:(jc % 4 + 1) * P]
                    else:
                        zb = ps.tile([P, 1024], F32, tag="big", bufs=2,
                                     name=f"zt{r}_{s}")
                        zsl = lambda jc: zb[:, jc * P:(jc + 1) * P]
                    # jc-outer: PSUM supports only ONE open accumulation chain
                    # per bank, so each jc's kt-chain must fully close before
                    # the next chain in the same bank starts. The sum chain
                    # lives in its own bank and may stay open throughout.
                    for jc in range(DT):
                        for h0, hcap in ((0, cap), (64, cap - 1)):
                            for kt in range(hcap):
                                nc.tensor.matmul(
                                    zsl(jc)[:, h0:h0 + 64],
                                    xnat_s[:, kt, jc * P:(jc + 1) * P],
                                    pt_s[:, OFF[s] + kt, h0:h0 + 64],
                                    start=(kt == 0), stop=(kt == hcap - 1))
                    for kt in range(cap):
                        nc.tensor.matmul(sum_ps[:, s:s + 1],
                                         pt_s[:, OFF[s] + kt, :], ones_s,
                                         start=(kt == 0), stop=(kt == cap - 1))
                    # copies after all chains: a copy overlapping later matmuls
                    # into the same tile would stall them (tile-granular deps)
                    for jc in range(DT):
                        eng = copy_dve if use_halves else cp_eng[jc % 2]
                        eng(out=zt_s[s % 2][:, jc, :], in_=zsl(jc))

                def emit_o(s, last=False):
                    rc = recip[s % 2]
                    nc.vector.reciprocal(out=rc, in_=sum_ps[:, s:s + 1])
                    if last:
                        # separate psum tiles per 512-chunk, ch-outer: ch0's
                        # divide+DMA overlap ch1's matmuls (deps are
                        # tile-granular, so a shared tile would serialize)
                        obs = [ps.tile([P, 512], F32, tag="half", bufs=3,
                                       name=f"o{r}_{s}_{i}") for i in range(2)]
                        osl = lambda ch: obs[ch]
                        loops = [(ch, jc) for ch in range(2) for jc in range(DT)]
                    else:
                        ob = ps.tile([P, 1024], F32, tag="big", bufs=2,
                                     name=f"o{r}_{s}")
                        osl = lambda ch: ob[:, ch * 512:(ch + 1) * 512]
                        loops = [(ch, jc) for jc in range(DT) for ch in range(2)]
                    for ch, jc in loops:
                        nc.tensor.matmul(
                            osl(ch),
                            zt_s[s % 2][:, jc, :],
                            wv_s[:, jc, ch * 512:(ch + 1) * 512],
                            start=(jc == 0), stop=(jc == DT - 1))
                        if last and ch == 0 and jc == DT - 1:
                            nc.scalar.mul(
                                out=o_sb[s % 2][:, 0:512], in_=osl(0), mul=rc)
                            nc.sync.dma_start(out=out_d[s * P:(s + 1) * P, 0:512],
                                              in_=o_sb[s % 2][:, 0:512])
                    for ch in ([1] if last else [0, 1]):
                        # divide on Act (per-partition scale), per 512-chunk
                        nc.scalar.mul(out=o_sb[s % 2][:, ch * 512:(ch + 1) * 512],
                                      in_=osl(ch), mul=rc)
                        nc.sync.dma_start(
                            out=out_d[s * P:(s + 1) * P, ch * 512:(ch + 1) * 512],
                            in_=o_sb[s % 2][:, ch * 512:(ch + 1) * 512])

                for s in range(8):
                    emit_st(s)
                    if s >= 1:
                        emit_zt(s - 1, use_halves=(s - 1 == 6))
                    if s >= 2:
                        emit_o(s - 2)
                emit_zt(7, use_halves=True)
                emit_o(6)
                emit_o(7, last=True)
                if "dbg" in EXP:
                    nc.sync.dma_start(out=dbg_m2, in_=m2_s)
                    nc.sync.dma_start(out=dbg_gt, in_=gt_s)
                    nc.sync.dma_start(out=dbg_pt, in_=pt_s)
                    nc.sync.dma_start(out=dbg_zt[:, 0], in_=zt_s[0])
                    nc.sync.dma_start(out=dbg_zt[:, 1], in_=zt_s[1])

    nc.compile()
    return nc


def _masks_for_core(h):
    """[128, 16*64] bf16: per slot s, diagonal masks for the even block
    (unit cap-1, cols 0:64) and odd block (unit cap-2, cols 64:128)."""
    bf = ml_dtypes.bfloat16
    m = np.zeros((P, 16 * 64), dtype=np.float32)
    kl = np.arange(P)[:, None]
    ql = np.arange(64)[None, :]
    for s, cap in enumerate(SLOT_CAPS):
        be, bo = B64[h][s]
        # even block diag: q = 64*be + ql vs keys k = 128*(cap-1) + kl
        m[:, 2 * s * 64:(2 * s + 1) * 64] = \
            (64 * be + ql >= 128 * (cap - 1) + kl).astype(np.float32)
        # odd block diag: unit cap-2
        m[:, (2 * s + 1) * 64:(2 * s + 2) * 64] = \
            (64 * bo + ql >= 128 * (cap - 2) + kl).astype(np.float32)
    return np.ascontiguousarray(m.astype(bf))


def _host_prep(x, Wq, Wk, Wv):
    """Build per-core input maps. x: [B,T,D] fp32."""
    bf = ml_dtypes.bfloat16
    WqT_b = np.ascontiguousarray(Wq.T.astype(bf))
    WkT_b = np.ascontiguousarray(Wk.T.astype(bf))
    Wv_b = np.ascontiguousarray(Wv.astype(bf))
    x_bf = x.astype(bf)                                  # [B, T, D]
    xT_by_batch = [np.ascontiguousarray(x_bf[b].T) for b in range(B)]
    masks_by_h = [_masks_for_core(0), _masks_for_core(1)]
    in_maps = []
    for c in range(8):
        b, h = divmod(c, 2)
        xb = x_bf[b]
        xq = np.concatenate(
            [xb[64 * g:64 * g + 64] for be_bo in B64[h] for g in be_bo], axis=0)
        in_maps.append({
            "WqT": WqT_b, "WkT": WkT_b, "Wv": Wv_b,
            "xqT": np.ascontiguousarray(xq.T),
            "xkT": xT_by_batch[b],
            "xnat": xb,
            "masks": masks_by_h[h],
        })
    return in_maps


def _reassemble(results, dtype=np.float32):
    out = np.empty((B, T, D), dtype=dtype)
    for c in range(8):
        b, h = divmod(c, 2)
        o = np.asarray(results[c]["out"], dtype=np.float32)  # [1024, D]
        for s, (be, bo) in enumerate(B64[h]):
            out[b, 64 * be:64 * be + 64] = o[s * P:s * P + 64]
            out[b, 64 * bo:64 * bo + 64] = o[s * P + 64:(s + 1) * P]
    return out


def kernel(**inputs):
    global _NC_CACHE
    x = np.asarray(inputs["x"], dtype=np.float32)
    Wq = np.asarray(inputs["Wq"], dtype=np.float32)
    Wk = np.asarray(inputs["Wk"], dtype=np.float32)
    Wv = np.asarray(inputs["Wv"], dtype=np.float32)
    if _NC_CACHE is None:
        _NC_CACHE = build_nc()
    nc = _NC_CACHE
    in_maps = _host_prep(x, Wq, Wk, Wv)
    res = run_bass_kernel_spmd(nc, in_maps, core_ids=list(range(8)))
    return _reassemble(res.results)


if __name__ == "__main__":
    rng = np.random.default_rng(0)
    x = rng.standard_normal((B, T, D), dtype=np.float32)
    Wq = rng.standard_normal((D, D), dtype=np.float32) / np.sqrt(D)
    Wk = rng.standard_normal((D, D), dtype=np.float32) / np.sqrt(D)
    Wv = rng.standard_normal((D, D), dtype=np.float32) / np.sqrt(D)
    out = kernel(x=x, Wq=Wq, Wk=Wk, Wv=Wv)
    print("out", out.shape, out.dtype, np.abs(out).max())

